# revision 2
# baseline (speedup 1.0000x reference)
"""Distributed multi-head attention kernel for 8 TRN2 NeuronCores.

Problem: hidden[2,2048,1024] -> QKV proj (16 heads, hd=64) -> softmax
attention -> out proj. f32 I/O, bf16 tensor-engine compute.

Sharding: sequence-parallel. Flattened rows [4096, 1024] split into 8
chunks of 512 rows; cores 0-3 own batch 0, cores 4-7 batch 1. Each core
projects K^T/V for its own chunk, AllGathers them within its 4-core
batch group, projects Q^T locally, then computes full 16-head attention
and the output projection for its 512 rows. No reduction collective is
needed: outputs are disjoint row blocks concatenated on the host.

Layouts (all on-chip compute in [dims, seq] "transposed" form so that
every matmul contraction sits on the partition axis):
  hT    [1024, 512]  hidden chunk, transposed on host, bf16
  kT    [1024, 512]  -> AllGather -> KTg [4096, 512] (4 chunks stacked)
  v     [512, 1024]  -> AllGather -> Vg  [2048, 1024] (keys in order)
  scoresT[keys, q]   per head: lhsT = KT slice [64, 128], rhs = qT [64, 512]
  probsT = exp(scoresT/8)   (no max subtraction: |scores| <~ 2 by
                             construction, softmax is shift-invariant)
  ctxT  [65, 512]    lhsT = [V_h | ones] so row 64 = softmax denominator
  out   [512, 1024]  lhsT = ctxT tiles, rhs = host-reordered Wo
"""

import numpy as np
import ml_dtypes

B, S, D, H, HD = 2, 2048, 1024, 16, 64
N_CORES = 8
ROWS = (B * S) // N_CORES          # 512 query rows per core
GROUP = 4                          # cores per batch group
P = 128
KT = D // P                        # 8 contraction tiles over hidden dim
KEYT = S // P                      # 16 key tiles per batch

_CACHE: dict = {}

bf16 = ml_dtypes.bfloat16


def _build_graph():
    import concourse.mybir as mybir
    import concourse.tile as tile
    from concourse import bacc
    from contextlib import ExitStack

    dt = mybir.dt
    F32, BF16 = dt.float32, dt.bfloat16
    AF = mybir.ActivationFunctionType

    nc = bacc.Bacc("TRN2", target_bir_lowering=False, debug=False,
                   enable_asserts=False, num_devices=N_CORES)

    hT = nc.dram_tensor("hT", [D, ROWS], BF16, kind="ExternalInput").ap()
    wq = nc.dram_tensor("wq", [D, D], BF16, kind="ExternalInput").ap()
    wk = nc.dram_tensor("wk", [D, D], BF16, kind="ExternalInput").ap()
    wv = nc.dram_tensor("wv", [D + 1, D], BF16, kind="ExternalInput").ap()
    wo = nc.dram_tensor("wo", [HD, H * D], BF16, kind="ExternalInput").ap()
    wob = nc.dram_tensor("wob", [1, D], BF16, kind="ExternalInput").ap()
    bqk = nc.dram_tensor("bqk", [P, 2 * KT], F32, kind="ExternalInput").ap()
    out = nc.dram_tensor("out", [ROWS, D], F32, kind="ExternalOutput").ap()

    with tile.TileContext(nc) as tc, ExitStack() as top:
        dram = top.enter_context(tc.tile_pool(name="dram", bufs=1, space="DRAM"))
        pers = top.enter_context(tc.tile_pool(name="pers", bufs=1))
        attn = top.enter_context(tc.tile_pool(name="attn", bufs=1))

        kb = dram.tile([D, ROWS], BF16)                 # local kT bounce
        vb = dram.tile([ROWS, D], BF16)                 # local v bounce
        KTg = dram.tile([GROUP * D, ROWS], BF16)        # gathered kT
        Vg = dram.tile([S, D], BF16)                    # gathered v

        # persistent small tensors
        ones_row = pers.tile([1, ROWS], BF16)
        nc.vector.memset(ones_row[:], 1.0)
        ones_b = pers.tile([P, HD], BF16)
        nc.vector.memset(ones_b[:], 1.0)
        bqk_sb = pers.tile([P, 2 * KT], F32)
        nc.sync.dma_start(bqk_sb[:], bqk[:])
        qT_sb = pers.tile([P, KT * ROWS], BF16)         # q^T, all dims

        # attention-phase big tensors (allocated up front; v_aug ones
        # columns are memset once, V data DMA'd over the first 64 cols)
        kt_sb = attn.tile([P, 4 * KT * ROWS], BF16)     # gathered K^T
        v_aug = attn.tile([P, KEYT * H * (HD + 1)], BF16)
        nc.gpsimd.memset(v_aug[:], 1.0)
        ctx_sb = attn.tile([HD, H * ROWS], BF16)        # normalized ctx^T
        wo_sb = attn.tile([HD, H * D], BF16)
        nc.sync.dma_start(wo_sb[:], wo[:])
        wob_sb = pers.tile([1, D], BF16)
        nc.sync.dma_start(wob_sb[:], wob[:])

        with ExitStack() as proj:
            wpool = proj.enter_context(tc.tile_pool(name="wpool", bufs=1))
            epool = proj.enter_context(tc.tile_pool(name="epool", bufs=3))
            ps_proj = proj.enter_context(
                tc.tile_pool(name="ps_proj", bufs=2, space="PSUM"))

            hT_sb = wpool.tile([P, KT * ROWS], BF16)
            nc.sync.dma_start(
                hT_sb[:].rearrange("p (k f) -> p k f", f=ROWS),
                hT[:].rearrange("(k p) f -> p k f", p=P))
            wk_sb = wpool.tile([P, KT * D], BF16)
            nc.sync.dma_start(
                wk_sb[:].rearrange("p (k f) -> p k f", f=D),
                wk[:].rearrange("(k p) f -> p k f", p=P))
            wv_sb = wpool.tile([P, KT * D], BF16)
            nc.sync.dma_start(
                wv_sb[:].rearrange("p (k f) -> p k f", f=D),
                wv[0:D, :].rearrange("(k p) f -> p k f", p=P))
            wv_b = wpool.tile([1, D], BF16)
            nc.sync.dma_start(wv_b[:], wv[D:D + 1, :])
            wq_sb = wpool.tile([P, KT * D], BF16)
            nc.sync.dma_start(
                wq_sb[:].rearrange("p (k f) -> p k f", f=D),
                wq[:].rearrange("(k p) f -> p k f", p=P))

            # k^T projection -> kb (bias bk folded into eviction)
            for m in range(KT):
                ps = ps_proj.tile([P, ROWS], F32, name="ps")
                for k in range(KT):
                    nc.tensor.matmul(
                        ps[:],
                        wk_sb[:, k * D + m * P: k * D + (m + 1) * P],
                        hT_sb[:, k * ROWS:(k + 1) * ROWS],
                        start=(k == 0), stop=(k == KT - 1))
                ev = epool.tile([P, ROWS], BF16, name="ev")
                nc.scalar.activation(ev[:], ps[:], AF.Identity,
                                     bias=bqk_sb[:, KT + m: KT + m + 1])
                nc.sync.dma_start(kb[m * P:(m + 1) * P, :], ev[:])

            nc.gpsimd.collective_compute(
                "AllGather", mybir.AluOpType.bypass,
                replica_groups=[[0, 1, 2, 3], [4, 5, 6, 7]],
                ins=[kb.opt()], outs=[KTg.opt()])

            # v projection -> vb (bias bv via ones-row matmul)
            for mk in range(ROWS // P):
                for n in range(D // 512):
                    ps = ps_proj.tile([P, 512], F32, name="ps")
                    for k in range(KT):
                        nc.tensor.matmul(
                            ps[:],
                            hT_sb[:, k * ROWS + mk * P: k * ROWS + (mk + 1) * P],
                            wv_sb[:, k * D + n * 512: k * D + (n + 1) * 512],
                            start=(k == 0), stop=False)
                    nc.tensor.matmul(
                        ps[:], ones_row[0:1, mk * P:(mk + 1) * P],
                        wv_b[0:1, n * 512:(n + 1) * 512],
                        start=False, stop=True)
                    ev = epool.tile([P, 512], BF16, name="ev")
                    nc.vector.tensor_copy(ev[:], ps[:])
                    nc.sync.dma_start(
                        vb[mk * P:(mk + 1) * P, n * 512:(n + 1) * 512], ev[:])

            nc.gpsimd.collective_compute(
                "AllGather", mybir.AluOpType.bypass,
                replica_groups=[[0, 1, 2, 3], [4, 5, 6, 7]],
                ins=[vb.opt()], outs=[Vg.opt()])

            # q^T projection -> qT_sb (bias bq folded into eviction)
            for m in range(KT):
                ps = ps_proj.tile([P, ROWS], F32, name="ps")
                for k in range(KT):
                    nc.tensor.matmul(
                        ps[:],
                        wq_sb[:, k * D + m * P: k * D + (m + 1) * P],
                        hT_sb[:, k * ROWS:(k + 1) * ROWS],
                        start=(k == 0), stop=(k == KT - 1))
                nc.scalar.activation(qT_sb[:, m * ROWS:(m + 1) * ROWS], ps[:],
                                     AF.Identity, bias=bqk_sb[:, m:m + 1])

        # gathered K^T / V into SBUF
        for r in range(GROUP):
            nc.sync.dma_start(
                kt_sb[:, r * KT * ROWS:(r + 1) * KT * ROWS]
                .rearrange("p (t f) -> p t f", f=ROWS),
                KTg[r * D:(r + 1) * D, :].rearrange("(t p) f -> p t f", p=P))
        for t in range(KEYT):
            base = t * H * (HD + 1)
            nc.sync.dma_start(
                v_aug[:, base: base + H * (HD + 1)]
                .rearrange("p (h a) -> p h a", a=HD + 1)[:, :, 0:HD],
                Vg[t * P:(t + 1) * P, :].rearrange("p (h d) -> p h d", d=HD))

        with ExitStack() as att:
            probs = att.enter_context(tc.tile_pool(name="probs", bufs=4))
            rpool = att.enter_context(tc.tile_pool(name="rpool", bufs=2))
            ps_s = att.enter_context(tc.tile_pool(name="ps_s", bufs=3, space="PSUM"))
            ps_ctx = att.enter_context(tc.tile_pool(name="ps_ctx", bufs=2, space="PSUM"))
            ps_b = att.enter_context(tc.tile_pool(name="ps_b", bufs=1, space="PSUM"))
            ps_o = att.enter_context(tc.tile_pool(name="ps_o", bufs=2, space="PSUM"))
            opool = att.enter_context(tc.tile_pool(name="opool", bufs=3))

            for h in range(H):
                jt, po = h // 2, (h % 2) * HD
                ps_c = ps_ctx.tile([HD + 1, ROWS], F32, name="ps_c")
                for t in range(KEYT):
                    r, m = t // (KEYT // GROUP), t % (KEYT // GROUP)
                    ps = ps_s.tile([P, ROWS], F32, name="ps_sc")
                    nc.tensor.matmul(
                        ps[:],
                        kt_sb[po:po + HD,
                              (r * KT + jt) * ROWS + m * P:
                              (r * KT + jt) * ROWS + (m + 1) * P],
                        qT_sb[po:po + HD, jt * ROWS:(jt + 1) * ROWS],
                        start=True, stop=True)
                    pt = probs.tile([P, ROWS], BF16, name="pt")
                    nc.scalar.activation(pt[:], ps[:], AF.Exp, scale=0.125)
                    off = (t * H + h) * (HD + 1)
                    nc.tensor.matmul(
                        ps_c[:], v_aug[:, off: off + HD + 1], pt[:],
                        start=(t == 0), stop=(t == KEYT - 1))
                # normalize: ctxT_h = ps_c[0:64] * bcast(1/ps_c[64])
                rb = rpool.tile([HD + 1, ROWS], BF16, name="rb")
                with nc.allow_low_precision("softmax denom recip in bf16"):
                    nc.vector.reciprocal(rb[HD:HD + 1, :], ps_c[HD:HD + 1, :])
                psb = ps_b.tile([HD, ROWS], F32, name="psb")
                nc.tensor.matmul(psb[:], ones_b[HD:HD + 1, 0:HD],
                                 rb[HD:HD + 1, :], start=True, stop=True)
                sbb = rpool.tile([HD, ROWS], BF16, name="sbb")
                nc.vector.tensor_copy(sbb[:], psb[:])
                nc.vector.tensor_mul(ctx_sb[:, h * ROWS:(h + 1) * ROWS],
                                     ps_c[0:HD, :], sbb[:])

            # output projection + bias bo
            for m in range(ROWS // P):
                for n in range(D // 512):
                    ps = ps_o.tile([P, 512], F32, name="ps_out")
                    for h in range(H):
                        nc.tensor.matmul(
                            ps[:],
                            ctx_sb[:, h * ROWS + m * P: h * ROWS + (m + 1) * P],
                            wo_sb[:, h * D + n * 512: h * D + (n + 1) * 512],
                            start=(h == 0), stop=False)
                    nc.tensor.matmul(
                        ps[:], ones_row[0:1, m * P:(m + 1) * P],
                        wob_sb[0:1, n * 512:(n + 1) * 512],
                        start=False, stop=True)
                    ot = opool.tile([P, 512], F32, name="ot")
                    nc.vector.tensor_copy(ot[:], ps[:])
                    nc.sync.dma_start(
                        out[m * P:(m + 1) * P, n * 512:(n + 1) * 512], ot[:])

    nc.compile()
    return nc


def _prep_inputs(hidden_states, Wq, bq, Wk, bk, Wv, bv, Wo, bo):
    hs = np.asarray(hidden_states, np.float32).reshape(B * S, D)
    wq = np.asarray(Wq, np.float32).astype(bf16)
    wk = np.asarray(Wk, np.float32).astype(bf16)
    wv = np.concatenate([np.asarray(Wv, np.float32),
                         np.asarray(bv, np.float32)[None]], 0).astype(bf16)
    wo = (np.asarray(Wo, np.float32).reshape(H, HD, D)
          .transpose(1, 0, 2).reshape(HD, H * D).astype(bf16))
    wob = np.asarray(bo, np.float32)[None].astype(bf16)
    bqk = np.concatenate([np.asarray(bq, np.float32).reshape(KT, P).T,
                          np.asarray(bk, np.float32).reshape(KT, P).T],
                         1).astype(np.float32)
    bqk = np.ascontiguousarray(bqk)
    in_maps = []
    for c in range(N_CORES):
        hT = np.ascontiguousarray(
            hs[c * ROWS:(c + 1) * ROWS].T).astype(bf16)
        in_maps.append({"hT": hT, "wq": wq, "wk": wk, "wv": wv,
                        "wo": wo, "wob": wob, "bqk": bqk})
    return in_maps


def _run(inputs, trace=False):
    from concourse import bass_utils
    if "nc" not in _CACHE:
        _CACHE["nc"] = _build_graph()
    nc = _CACHE["nc"]
    in_maps = _prep_inputs(**inputs)
    res = bass_utils.run_bass_kernel_spmd(
        nc, in_maps, core_ids=list(range(N_CORES)), trace=trace)
    full = np.concatenate([res.results[c]["out"] for c in range(N_CORES)],
                          axis=0).reshape(B, S, D).astype(np.float32)
    return full, res


def kernel(**inputs) -> np.ndarray:
    full, _ = _run(inputs, trace=False)
    return full


# revision 6
# speedup vs baseline: 1.1804x; 1.1804x over previous
"""Distributed multi-head attention kernel for 8 TRN2 NeuronCores.

Problem: hidden[2,2048,1024] -> QKV proj (16 heads, hd=64) -> softmax
attention -> out proj. f32 I/O, bf16 tensor-engine compute.

Sharding: sequence-parallel. Flattened rows [4096, 1024] split into 8
chunks of 512 rows; cores 0-3 own batch 0, cores 4-7 batch 1. Each core
projects K^T for its own chunk and AllGathers it within its 4-core
batch group (hidden under compute); V is projected redundantly for the
full batch on every core (cheaper than a second, serialized AllGather),
written directly into SBUF. Q^T is local. Each core then runs full
16-head attention + output projection for its 512 rows; outputs are
disjoint row blocks concatenated on the host — no reduction collective.

Layouts (all compute in [dims, seq] "transposed" form so matmul
contractions sit on the partition axis):
  hT_own [1024, 512], hT_full [1024, 2048] (host-transposed, bf16)
  kT [1024, 512] -> AllGather -> KTg [4096, 512]
  scoresT [keys, q] per head; probsT = exp(scoresT/8) (no max
    subtraction: |scores| <~ 2 by construction, softmax shift-invariant)
  ctxT accumulates via lhsT = [V_h | ones]: psum [65, 512], row 64 =
    softmax denominator; normalize = approx-reciprocal + gpsimd
    partition-broadcast + DVE multiply
  out [512, 1024] via lhsT = ctxT tiles, rhs = host-reordered Wo
"""

import numpy as np
import ml_dtypes

B, S, D, H, HD = 2, 2048, 1024, 16, 64
N_CORES = 8
ROWS = (B * S) // N_CORES          # 512 query rows per core
GROUP = 4                          # cores per batch group
P = 128
KT = D // P                        # 8 contraction tiles over hidden dim
KEYT = S // P                      # 16 key tiles per batch
HA = HD + 1                        # head slot width in v_aug

_CACHE: dict = {}

bf16 = ml_dtypes.bfloat16


def _build_graph():
    import concourse.mybir as mybir
    import concourse.tile as tile
    from concourse import bacc
    from contextlib import ExitStack

    dt = mybir.dt
    F32, BF16 = dt.float32, dt.bfloat16
    AF = mybir.ActivationFunctionType

    nc = bacc.Bacc("TRN2", target_bir_lowering=False, debug=False,
                   enable_asserts=False, num_devices=N_CORES)

    hT = nc.dram_tensor("hT", [D, ROWS], BF16, kind="ExternalInput").ap()
    hTf = nc.dram_tensor("hTf", [D, S], BF16, kind="ExternalInput").ap()
    wq = nc.dram_tensor("wq", [D, D], BF16, kind="ExternalInput").ap()
    wk = nc.dram_tensor("wk", [D, D], BF16, kind="ExternalInput").ap()
    wv = nc.dram_tensor("wv", [D + 1, D], BF16, kind="ExternalInput").ap()
    wo = nc.dram_tensor("wo", [HD, H * D], BF16, kind="ExternalInput").ap()
    wob = nc.dram_tensor("wob", [1, D], BF16, kind="ExternalInput").ap()
    bqk = nc.dram_tensor("bqk", [P, 2 * KT], F32, kind="ExternalInput").ap()
    out = nc.dram_tensor("out", [ROWS, D], F32, kind="ExternalOutput").ap()

    with tile.TileContext(nc) as tc, ExitStack() as top:
        dram = top.enter_context(tc.tile_pool(name="dram", bufs=1, space="DRAM"))
        pers = top.enter_context(tc.tile_pool(name="pers", bufs=1))
        attn = top.enter_context(tc.tile_pool(name="attn", bufs=1))

        kb = dram.tile([D, ROWS], BF16)                 # local kT bounce
        KTg = dram.tile([GROUP * D, ROWS], BF16)        # gathered kT

        ones_row = pers.tile([1, S], BF16)
        nc.vector.memset(ones_row[:], 1.0)
        ones_b = pers.tile([P, HD], BF16)
        nc.vector.memset(ones_b[:], 1.0)
        bqk_sb = pers.tile([P, 2 * KT], F32)
        nc.sync.dma_start(bqk_sb[:], bqk[:])
        qT_sb = pers.tile([P, KT * ROWS], BF16)         # q^T, all dims

        kt_sb = attn.tile([P, 4 * KT * ROWS], BF16)     # gathered K^T
        v_aug = attn.tile([P, KEYT * H * HA], BF16)     # [V_h | 1] slots
        nc.gpsimd.memset(v_aug[:], 1.0)
        ctx_sb = attn.tile([HD, H * ROWS], BF16)        # normalized ctx^T

        with ExitStack() as proj:
            wpool = proj.enter_context(tc.tile_pool(name="wpool", bufs=1))
            epool = proj.enter_context(tc.tile_pool(name="epool", bufs=3))
            ps_proj = proj.enter_context(
                tc.tile_pool(name="ps_proj", bufs=3, space="PSUM"))

            # input DMAs, most-urgent first
            wk_sb = wpool.tile([P, KT * D], BF16)
            nc.sync.dma_start(
                wk_sb[:].rearrange("p (k f) -> p k f", f=D),
                wk[:].rearrange("(k p) f -> p k f", p=P))
            hT_sb = wpool.tile([P, KT * ROWS], BF16)
            nc.sync.dma_start(
                hT_sb[:].rearrange("p (k f) -> p k f", f=ROWS),
                hT[:].rearrange("(k p) f -> p k f", p=P))
            hTf_sb = wpool.tile([P, KT * S], BF16)
            nc.sync.dma_start(
                hTf_sb[:].rearrange("p (k f) -> p k f", f=S),
                hTf[:].rearrange("(k p) f -> p k f", p=P))
            wv_sb = wpool.tile([P, KT * D], BF16)
            nc.sync.dma_start(
                wv_sb[:].rearrange("p (k f) -> p k f", f=D),
                wv[0:D, :].rearrange("(k p) f -> p k f", p=P))
            wv_b = wpool.tile([1, D], BF16)
            nc.sync.dma_start(wv_b[:], wv[D:D + 1, :])
            wq_sb = wpool.tile([P, KT * D], BF16)
            nc.sync.dma_start(
                wq_sb[:].rearrange("p (k f) -> p k f", f=D),
                wq[:].rearrange("(k p) f -> p k f", p=P))

            # k^T projection -> kb (bias bk folded into eviction)
            for m in range(KT):
                ps = ps_proj.tile([P, ROWS], F32, name="ps")
                for k in range(KT):
                    nc.tensor.matmul(
                        ps[:],
                        wk_sb[:, k * D + m * P: k * D + (m + 1) * P],
                        hT_sb[:, k * ROWS:(k + 1) * ROWS],
                        start=(k == 0), stop=(k == KT - 1))
                ev = epool.tile([P, ROWS], BF16, name="ev")
                nc.scalar.activation(ev[:], ps[:], AF.Identity,
                                     bias=bqk_sb[:, KT + m: KT + m + 1])
                nc.sync.dma_start(kb[m * P:(m + 1) * P, :], ev[:])

            nc.gpsimd.collective_compute(
                "AllGather", mybir.AluOpType.bypass,
                replica_groups=[[0, 1, 2, 3], [4, 5, 6, 7]],
                ins=[kb.opt()], outs=[KTg.opt()])

            # full-batch V projection, evicted straight into v_aug slots
            for mk in range(KEYT):
                for n in range(2):
                    ps = ps_proj.tile([P, 512], F32, name="ps")
                    for k in range(KT):
                        nc.tensor.matmul(
                            ps[:],
                            hTf_sb[:, k * S + mk * P: k * S + (mk + 1) * P],
                            wv_sb[:, k * D + n * 512: k * D + (n + 1) * 512],
                            start=(k == 0), stop=False)
                    nc.tensor.matmul(
                        ps[:], ones_row[0:1, mk * P:(mk + 1) * P],
                        wv_b[0:1, n * 512:(n + 1) * 512],
                        start=False, stop=True)
                    base = (mk * H + n * 8) * HA
                    nc.vector.tensor_copy(
                        v_aug[:, base: base + 8 * HA]
                        .rearrange("p (h a) -> p h a", a=HA)[:, :, 0:HD],
                        ps[:].rearrange("p (h d) -> p h d", d=HD))

            # q^T projection -> qT_sb (bias bq folded into eviction)
            for m in range(KT):
                ps = ps_proj.tile([P, ROWS], F32, name="ps")
                for k in range(KT):
                    nc.tensor.matmul(
                        ps[:],
                        wq_sb[:, k * D + m * P: k * D + (m + 1) * P],
                        hT_sb[:, k * ROWS:(k + 1) * ROWS],
                        start=(k == 0), stop=(k == KT - 1))
                nc.scalar.activation(qT_sb[:, m * ROWS:(m + 1) * ROWS], ps[:],
                                     AF.Identity, bias=bqk_sb[:, m:m + 1])

        # gathered K^T into SBUF
        for r in range(GROUP):
            nc.sync.dma_start(
                kt_sb[:, r * KT * ROWS:(r + 1) * KT * ROWS]
                .rearrange("p (t f) -> p t f", f=ROWS),
                KTg[r * D:(r + 1) * D, :].rearrange("(t p) f -> p t f", p=P))

        with ExitStack() as att:
            late = att.enter_context(tc.tile_pool(name="late", bufs=1))
            wo_sb = late.tile([HD, H * D], BF16)
            nc.sync.dma_start(wo_sb[:], wo[:])
            wob_sb = late.tile([1, D], BF16)
            nc.sync.dma_start(wob_sb[:], wob[:])

            probs = att.enter_context(tc.tile_pool(name="probs", bufs=12))
            rpool = att.enter_context(tc.tile_pool(name="rpool", bufs=2))
            bpool = att.enter_context(tc.tile_pool(name="bpool", bufs=2))

            with ExitStack() as attp:
                ps_s = attp.enter_context(
                    tc.tile_pool(name="ps_s", bufs=2, space="PSUM"))
                ps_ctx = attp.enter_context(
                    tc.tile_pool(name="ps_ctx", bufs=3, space="PSUM"))
                ps_b = attp.enter_context(
                    tc.tile_pool(name="ps_b", bufs=1, space="PSUM"))

                # software-pipelined attention: head pairs (p), u-steps of
                # two key tiles; ctx/exp consumption lags scores by LAG_U.
                U = KEYT // 2
                LAG_U = 4
                pend = {}
                psc = {}

                def emit_scores(p, u):
                    for hh in (0, 1):
                        h = 2 * p + hh
                        jt, po = h // 2, (h % 2) * HD
                        ps = ps_s.tile([P, 2 * ROWS], F32, name="ps_sc")
                        for half in (0, 1):
                            t = 2 * u + half
                            r, m = t // (KEYT // GROUP), t % (KEYT // GROUP)
                            nc.tensor.matmul(
                                ps[:, half * ROWS:(half + 1) * ROWS],
                                kt_sb[po:po + HD,
                                      (r * KT + jt) * ROWS + m * P:
                                      (r * KT + jt) * ROWS + (m + 1) * P],
                                qT_sb[po:po + HD, jt * ROWS:(jt + 1) * ROWS],
                                start=True, stop=True)
                        pt = probs.tile([P, 2 * ROWS], BF16, name="pt")
                        nc.scalar.activation(pt[:], ps[:], AF.Exp, scale=0.125)
                        pend[(h, u)] = pt

                def emit_ctx(p, u):
                    for hh in (0, 1):
                        h = 2 * p + hh
                        if u == 0:
                            psc[h] = ps_ctx.tile([HA, ROWS], F32, name="ps_c")
                        pt = pend.pop((h, u))
                        for half in (0, 1):
                            t = 2 * u + half
                            off = (t * H + h) * HA
                            nc.tensor.matmul(
                                psc[h][:], v_aug[:, off: off + HA],
                                pt[:, half * ROWS:(half + 1) * ROWS],
                                start=(t == 0), stop=(t == KEYT - 1))
                        if u == U - 1:
                            ps_c = psc.pop(h)
                            rt = rpool.tile([HA, ROWS], BF16, name="rt")
                            with nc.allow_low_precision("softmax denom recip"):
                                nc.vector.reciprocal(
                                    rt[HD:HD + 1, :], ps_c[HD:HD + 1, :])
                            psb = ps_b.tile([HD, ROWS], F32, name="psb")
                            nc.tensor.matmul(psb[:], ones_b[HD:HD + 1, 0:HD],
                                             rt[HD:HD + 1, :],
                                             start=True, stop=True)
                            sbb = bpool.tile([HD, ROWS], BF16, name="sbb")
                            nc.vector.tensor_copy(sbb[:], psb[:])
                            nc.vector.tensor_mul(
                                ctx_sb[:, h * ROWS:(h + 1) * ROWS],
                                ps_c[0:HD, :], sbb[:])

                NP = H // 2
                for G in range(NP * U + LAG_U):
                    if G < NP * U:
                        emit_scores(*divmod(G, U))
                    if G >= LAG_U:
                        emit_ctx(*divmod(G - LAG_U, U))

            with ExitStack() as outp_s:
                ps_o = outp_s.enter_context(
                    tc.tile_pool(name="ps_o", bufs=2, space="PSUM"))
                opool = outp_s.enter_context(tc.tile_pool(name="opool", bufs=3))
                for m in range(ROWS // P):
                    for n in range(2):
                        ps = ps_o.tile([P, 512], F32, name="ps_out")
                        for h in range(H):
                            nc.tensor.matmul(
                                ps[:],
                                ctx_sb[:, h * ROWS + m * P:
                                       h * ROWS + (m + 1) * P],
                                wo_sb[:, h * D + n * 512: h * D + (n + 1) * 512],
                                start=(h == 0), stop=False)
                        nc.tensor.matmul(
                            ps[:], ones_row[0:1, m * P:(m + 1) * P],
                            wob_sb[0:1, n * 512:(n + 1) * 512],
                            start=False, stop=True)
                        ot = opool.tile([P, 512], F32, name="ot")
                        nc.vector.tensor_copy(ot[:], ps[:])
                        nc.sync.dma_start(
                            out[m * P:(m + 1) * P, n * 512:(n + 1) * 512],
                            ot[:])

    nc.compile()
    return nc


def _prep_inputs(hidden_states, Wq, bq, Wk, bk, Wv, bv, Wo, bo):
    hs = np.asarray(hidden_states, np.float32).reshape(B * S, D)
    wq = np.asarray(Wq, np.float32).astype(bf16)
    wk = np.asarray(Wk, np.float32).astype(bf16)
    wv = np.concatenate([np.asarray(Wv, np.float32),
                         np.asarray(bv, np.float32)[None]], 0).astype(bf16)
    wo = (np.asarray(Wo, np.float32).reshape(H, HD, D)
          .transpose(1, 0, 2).reshape(HD, H * D).astype(bf16))
    wob = np.asarray(bo, np.float32)[None].astype(bf16)
    bqk = np.ascontiguousarray(np.concatenate(
        [np.asarray(bq, np.float32).reshape(KT, P).T,
         np.asarray(bk, np.float32).reshape(KT, P).T], 1).astype(np.float32))
    hTf = [np.ascontiguousarray(hs[b * S:(b + 1) * S].T).astype(bf16)
           for b in range(B)]
    in_maps = []
    for c in range(N_CORES):
        hT = np.ascontiguousarray(
            hs[c * ROWS:(c + 1) * ROWS].T).astype(bf16)
        in_maps.append({"hT": hT, "hTf": hTf[c // GROUP], "wq": wq, "wk": wk,
                        "wv": wv, "wo": wo, "wob": wob, "bqk": bqk})
    return in_maps


def _run(inputs, trace=False):
    from concourse import bass_utils
    if "nc" not in _CACHE:
        _CACHE["nc"] = _build_graph()
    nc = _CACHE["nc"]
    in_maps = _prep_inputs(**inputs)
    res = bass_utils.run_bass_kernel_spmd(
        nc, in_maps, core_ids=list(range(N_CORES)), trace=trace)
    full = np.concatenate([res.results[c]["out"] for c in range(N_CORES)],
                          axis=0).reshape(B, S, D).astype(np.float32)
    return full, res


def kernel(**inputs) -> np.ndarray:
    full, _ = _run(inputs, trace=False)
    return full


# revision 11
# speedup vs baseline: 1.2011x; 1.0175x over previous
"""Distributed multi-head attention kernel for 8 TRN2 NeuronCores.

Problem: hidden[2,2048,1024] -> QKV proj (16 heads, hd=64) -> softmax
attention -> out proj. f32 I/O, bf16 tensor-engine compute.

Sharding: sequence-parallel. Flattened rows [4096, 1024] split into 8
chunks of 512 rows; cores 0-3 own batch 0, cores 4-7 batch 1. Each core
projects K^T for its own chunk and AllGathers it within its 4-core
batch group (hidden under compute); V is projected redundantly for the
full batch on every core (cheaper than a second, serialized AllGather),
written directly into SBUF. Q^T is local. Each core then runs full
16-head attention + output projection for its 512 rows; outputs are
disjoint row blocks concatenated on the host — no reduction collective.

Layouts (all compute in [dims, seq] "transposed" form so matmul
contractions sit on the partition axis):
  hT_own [1024, 512], hT_full [1024, 2048] (host-transposed, bf16)
  kT [1024, 512] -> AllGather -> KTg [4096, 512]
  scoresT [keys, q] per head; probsT = exp(scoresT/8) (no max
    subtraction: |scores| <~ 2 by construction, softmax shift-invariant)
  ctxT accumulates via lhsT = [V_h | ones]: psum [65, 512], row 64 =
    softmax denominator; normalize = approx-reciprocal + gpsimd
    partition-broadcast + DVE multiply
  out [512, 1024] via lhsT = ctxT tiles, rhs = host-reordered Wo
"""

import numpy as np
import ml_dtypes

B, S, D, H, HD = 2, 2048, 1024, 16, 64
N_CORES = 8
ROWS = (B * S) // N_CORES          # 512 query rows per core
GROUP = 4                          # cores per batch group
P = 128
KT = D // P                        # 8 contraction tiles over hidden dim
KEYT = S // P                      # 16 key tiles per batch
HA = HD + 1                        # head slot width in v_aug

_CACHE: dict = {}

bf16 = ml_dtypes.bfloat16


def _build_graph():
    import concourse.mybir as mybir
    import concourse.tile as tile
    from concourse import bacc
    from contextlib import ExitStack

    dt = mybir.dt
    F32, BF16 = dt.float32, dt.bfloat16
    AF = mybir.ActivationFunctionType

    nc = bacc.Bacc("TRN2", target_bir_lowering=False, debug=False,
                   enable_asserts=False, num_devices=N_CORES)

    hT = nc.dram_tensor("hT", [D, ROWS], BF16, kind="ExternalInput").ap()
    hTf = nc.dram_tensor("hTf", [D, S], BF16, kind="ExternalInput").ap()
    wq = nc.dram_tensor("wq", [D, D], BF16, kind="ExternalInput").ap()
    wk = nc.dram_tensor("wk", [D, D], BF16, kind="ExternalInput").ap()
    wv = nc.dram_tensor("wv", [D + 1, D], BF16, kind="ExternalInput").ap()
    wo = nc.dram_tensor("wo", [HD, H * D], BF16, kind="ExternalInput").ap()
    wob = nc.dram_tensor("wob", [1, D], BF16, kind="ExternalInput").ap()
    bqk = nc.dram_tensor("bqk", [P, 2 * KT], F32, kind="ExternalInput").ap()
    out = nc.dram_tensor("out", [ROWS, D], F32, kind="ExternalOutput").ap()

    with tile.TileContext(nc) as tc, ExitStack() as top:
        dram = top.enter_context(tc.tile_pool(name="dram", bufs=1, space="DRAM"))
        pers = top.enter_context(tc.tile_pool(name="pers", bufs=1))
        attn = top.enter_context(tc.tile_pool(name="attn", bufs=1))

        kb = dram.tile([D, ROWS], BF16)                 # local kT bounce
        KTg = dram.tile([GROUP * D, ROWS], BF16)        # gathered kT

        ones_row = pers.tile([1, S], BF16)
        nc.vector.memset(ones_row[:], 1.0)
        ones_b = pers.tile([P, HD], BF16)
        nc.vector.memset(ones_b[:], 1.0)
        bqk_sb = pers.tile([P, 2 * KT], F32)
        nc.sync.dma_start(bqk_sb[:], bqk[:])
        qT_sb = pers.tile([P, KT * ROWS], BF16)         # q^T, all dims

        kt_sb = attn.tile([P, 4 * KT * ROWS], BF16)     # gathered K^T
        v_aug = attn.tile([P, KEYT * H * HA], BF16)     # [V_h | 1] slots
        nc.gpsimd.memset(v_aug[:], 1.0)
        ctx_sb = attn.tile([HD, H * ROWS], BF16)        # normalized ctx^T

        with ExitStack() as proj:
            wpool = proj.enter_context(tc.tile_pool(name="wpool", bufs=1))
            epool = proj.enter_context(tc.tile_pool(name="epool", bufs=3))
            ps_proj = proj.enter_context(
                tc.tile_pool(name="ps_proj", bufs=3, space="PSUM"))

            # input DMAs, most-urgent first
            wk_sb = wpool.tile([P, KT * D], BF16)
            nc.sync.dma_start(
                wk_sb[:].rearrange("p (k f) -> p k f", f=D),
                wk[:].rearrange("(k p) f -> p k f", p=P))
            hT_sb = wpool.tile([P, KT * ROWS], BF16)
            nc.sync.dma_start(
                hT_sb[:].rearrange("p (k f) -> p k f", f=ROWS),
                hT[:].rearrange("(k p) f -> p k f", p=P))
            wv_sb = wpool.tile([P, KT * D], BF16)
            nc.sync.dma_start(
                wv_sb[:].rearrange("p (k f) -> p k f", f=D),
                wv[0:D, :].rearrange("(k p) f -> p k f", p=P))
            wv_b = wpool.tile([1, D], BF16)
            nc.sync.dma_start(wv_b[:], wv[D:D + 1, :])
            hTf_sb = wpool.tile([P, KT * S], BF16)
            for kk in range(4):
                nc.sync.dma_start(
                    hTf_sb[:, kk * 2 * S:(kk + 1) * 2 * S]
                    .rearrange("p (k f) -> p k f", f=S),
                    hTf[kk * 2 * P:(kk + 1) * 2 * P, :]
                    .rearrange("(k p) f -> p k f", p=P))
            wq_sb = wpool.tile([P, KT * D], BF16)
            nc.sync.dma_start(
                wq_sb[:].rearrange("p (k f) -> p k f", f=D),
                wq[:].rearrange("(k p) f -> p k f", p=P))

            # k^T projection -> kb (bias bk folded into eviction)
            for m in range(KT):
                ps = ps_proj.tile([P, ROWS], F32, name="ps")
                for k in range(KT):
                    nc.tensor.matmul(
                        ps[:],
                        wk_sb[:, k * D + m * P: k * D + (m + 1) * P],
                        hT_sb[:, k * ROWS:(k + 1) * ROWS],
                        start=(k == 0), stop=(k == KT - 1))
                ev = epool.tile([P, ROWS], BF16, name="ev")
                nc.scalar.activation(ev[:], ps[:], AF.Identity,
                                     bias=bqk_sb[:, KT + m: KT + m + 1])
                nc.sync.dma_start(kb[m * P:(m + 1) * P, :], ev[:])

            nc.gpsimd.collective_compute(
                "AllGather", mybir.AluOpType.bypass,
                replica_groups=[[0, 1, 2, 3], [4, 5, 6, 7]],
                ins=[kb.opt()], outs=[KTg.opt()])

            # full-batch V projection, evicted straight into v_aug slots
            for mk in range(KEYT):
                for n in range(2):
                    ps = ps_proj.tile([P, 512], F32, name="ps")
                    for k in range(KT):
                        nc.tensor.matmul(
                            ps[:],
                            hTf_sb[:, k * S + mk * P: k * S + (mk + 1) * P],
                            wv_sb[:, k * D + n * 512: k * D + (n + 1) * 512],
                            start=(k == 0), stop=False)
                    nc.tensor.matmul(
                        ps[:], ones_row[0:1, mk * P:(mk + 1) * P],
                        wv_b[0:1, n * 512:(n + 1) * 512],
                        start=False, stop=True)
                    base = (mk * H + n * 8) * HA
                    nc.vector.tensor_copy(
                        v_aug[:, base: base + 8 * HA]
                        .rearrange("p (h a) -> p h a", a=HA)[:, :, 0:HD],
                        ps[:].rearrange("p (h d) -> p h d", d=HD))

            # q^T projection -> qT_sb (bias bq folded into eviction)
            for m in range(KT):
                ps = ps_proj.tile([P, ROWS], F32, name="ps")
                for k in range(KT):
                    nc.tensor.matmul(
                        ps[:],
                        wq_sb[:, k * D + m * P: k * D + (m + 1) * P],
                        hT_sb[:, k * ROWS:(k + 1) * ROWS],
                        start=(k == 0), stop=(k == KT - 1))
                nc.scalar.activation(qT_sb[:, m * ROWS:(m + 1) * ROWS], ps[:],
                                     AF.Identity, bias=bqk_sb[:, m:m + 1])

        # gathered K^T into SBUF
        for r in range(GROUP):
            nc.sync.dma_start(
                kt_sb[:, r * KT * ROWS:(r + 1) * KT * ROWS]
                .rearrange("p (t f) -> p t f", f=ROWS),
                KTg[r * D:(r + 1) * D, :].rearrange("(t p) f -> p t f", p=P))

        with ExitStack() as att:
            late = att.enter_context(tc.tile_pool(name="late", bufs=1))
            wo_sb = late.tile([HD, H * D], BF16)
            nc.sync.dma_start(wo_sb[:], wo[:])
            wob_sb = late.tile([1, D], BF16)
            nc.sync.dma_start(wob_sb[:], wob[:])

            probs = att.enter_context(tc.tile_pool(name="probs", bufs=12))
            rpool = att.enter_context(tc.tile_pool(name="rpool", bufs=2))
            bpool = att.enter_context(tc.tile_pool(name="bpool", bufs=2))

            with ExitStack() as attp:
                ps_s = attp.enter_context(
                    tc.tile_pool(name="ps_s", bufs=2, space="PSUM"))
                ps_ctx = attp.enter_context(
                    tc.tile_pool(name="ps_ctx", bufs=3, space="PSUM"))
                ps_b = attp.enter_context(
                    tc.tile_pool(name="ps_b", bufs=1, space="PSUM"))

                # software-pipelined attention: head pairs (p), u-steps of
                # two key tiles; ctx/exp consumption lags scores by LAG_U.
                U = KEYT // 2
                LAG_U = 4
                pend = {}
                psc = {}

                norm_q = []

                def emit_scores(p, u):
                    # alternate the two heads' row-groups (even head rows
                    # 0-63, odd rows 64-127) so adjacent K=64 matmuls can
                    # execute concurrently in the PE array
                    tiles = []
                    for hh in (0, 1):
                        tiles.append(ps_s.tile([P, 2 * ROWS], F32,
                                               name="ps_sc"))
                    for half in (0, 1):
                        t = 2 * u + half
                        r, m = t // (KEYT // GROUP), t % (KEYT // GROUP)
                        for hh in (0, 1):
                            h = 2 * p + hh
                            jt, po = h // 2, (h % 2) * HD
                            nc.tensor.matmul(
                                tiles[hh][:, half * ROWS:(half + 1) * ROWS],
                                kt_sb[po:po + HD,
                                      (r * KT + jt) * ROWS + m * P:
                                      (r * KT + jt) * ROWS + (m + 1) * P],
                                qT_sb[po:po + HD, jt * ROWS:(jt + 1) * ROWS],
                                start=True, stop=True)
                    for hh in (0, 1):
                        pt = probs.tile([P, 2 * ROWS], BF16, name="pt")
                        nc.scalar.activation(pt[:], tiles[hh][:], AF.Exp,
                                             scale=0.125)
                        pend[(2 * p + hh, u)] = pt

                def emit_ctx(G, p, u):
                    for hh in (0, 1):
                        h = 2 * p + hh
                        if u == 0:
                            psc[h] = ps_ctx.tile([HA, ROWS], F32, name="ps_c")
                        pt = pend.pop((h, u))
                        for half in (0, 1):
                            t = 2 * u + half
                            off = (t * H + h) * HA
                            nc.tensor.matmul(
                                psc[h][:], v_aug[:, off: off + HA],
                                pt[:, half * ROWS:(half + 1) * ROWS],
                                start=(t == 0), stop=(t == KEYT - 1))
                        if u == U - 1:
                            # reciprocal starts now (DVE); the dependent PE
                            # broadcast-matmul is deferred NORM_LAG G-steps
                            # so the PE never waits on the reciprocal
                            ps_c = psc.pop(h)
                            rt = rpool.tile([HA, ROWS], BF16, name="rt")
                            with nc.allow_low_precision("softmax denom recip"):
                                nc.vector.reciprocal(
                                    rt[HD:HD + 1, :], ps_c[HD:HD + 1, :])
                            norm_q.append((G, h, ps_c, rt))

                def emit_norm():
                    _, h, ps_c, rt = norm_q.pop(0)
                    psb = ps_b.tile([HD, ROWS], F32, name="psb")
                    nc.tensor.matmul(psb[:], ones_b[HD:HD + 1, 0:HD],
                                     rt[HD:HD + 1, :], start=True, stop=True)
                    sbb = bpool.tile([HD, ROWS], BF16, name="sbb")
                    nc.vector.tensor_copy(sbb[:], psb[:])
                    nc.vector.tensor_mul(
                        ctx_sb[:, h * ROWS:(h + 1) * ROWS],
                        ps_c[0:HD, :], sbb[:])

                NP = H // 2
                NORM_LAG = 1
                for G in range(NP * U + LAG_U + NORM_LAG + 1):
                    # pop pending normalizes first so their ps_ctx slots free
                    # before this G-step's emit_ctx may allocate new ones
                    while norm_q and G - norm_q[0][0] >= NORM_LAG:
                        emit_norm()
                    if G < NP * U:
                        emit_scores(*divmod(G, U))
                    if LAG_U <= G < NP * U + LAG_U:
                        emit_ctx(G, *divmod(G - LAG_U, U))

            with ExitStack() as outp_s:
                ps_o = outp_s.enter_context(
                    tc.tile_pool(name="ps_o", bufs=2, space="PSUM"))
                opool = outp_s.enter_context(tc.tile_pool(name="opool", bufs=3))
                for m in range(ROWS // P):
                    for n in range(2):
                        ps = ps_o.tile([P, 512], F32, name="ps_out")
                        for h in range(H):
                            nc.tensor.matmul(
                                ps[:],
                                ctx_sb[:, h * ROWS + m * P:
                                       h * ROWS + (m + 1) * P],
                                wo_sb[:, h * D + n * 512: h * D + (n + 1) * 512],
                                start=(h == 0), stop=False)
                        nc.tensor.matmul(
                            ps[:], ones_row[0:1, m * P:(m + 1) * P],
                            wob_sb[0:1, n * 512:(n + 1) * 512],
                            start=False, stop=True)
                        ot = opool.tile([P, 512], F32, name="ot")
                        nc.vector.tensor_copy(ot[:], ps[:])
                        nc.sync.dma_start(
                            out[m * P:(m + 1) * P, n * 512:(n + 1) * 512],
                            ot[:])

    nc.compile()
    return nc


def _prep_inputs(hidden_states, Wq, bq, Wk, bk, Wv, bv, Wo, bo):
    hs = np.asarray(hidden_states, np.float32).reshape(B * S, D)
    wq = np.asarray(Wq, np.float32).astype(bf16)
    wk = np.asarray(Wk, np.float32).astype(bf16)
    wv = np.concatenate([np.asarray(Wv, np.float32),
                         np.asarray(bv, np.float32)[None]], 0).astype(bf16)
    wo = (np.asarray(Wo, np.float32).reshape(H, HD, D)
          .transpose(1, 0, 2).reshape(HD, H * D).astype(bf16))
    wob = np.asarray(bo, np.float32)[None].astype(bf16)
    bqk = np.ascontiguousarray(np.concatenate(
        [np.asarray(bq, np.float32).reshape(KT, P).T,
         np.asarray(bk, np.float32).reshape(KT, P).T], 1).astype(np.float32))
    hTf = [np.ascontiguousarray(hs[b * S:(b + 1) * S].T).astype(bf16)
           for b in range(B)]
    in_maps = []
    for c in range(N_CORES):
        hT = np.ascontiguousarray(
            hs[c * ROWS:(c + 1) * ROWS].T).astype(bf16)
        in_maps.append({"hT": hT, "hTf": hTf[c // GROUP], "wq": wq, "wk": wk,
                        "wv": wv, "wo": wo, "wob": wob, "bqk": bqk})
    return in_maps


def _run(inputs, trace=False):
    from concourse import bass_utils
    if "nc" not in _CACHE:
        _CACHE["nc"] = _build_graph()
    nc = _CACHE["nc"]
    in_maps = _prep_inputs(**inputs)
    res = bass_utils.run_bass_kernel_spmd(
        nc, in_maps, core_ids=list(range(N_CORES)), trace=trace)
    full = np.concatenate([res.results[c]["out"] for c in range(N_CORES)],
                          axis=0).reshape(B, S, D).astype(np.float32)
    return full, res


def kernel(**inputs) -> np.ndarray:
    full, _ = _run(inputs, trace=False)
    return full


# revision 22
# speedup vs baseline: 1.5019x; 1.2504x over previous
"""Distributed multi-head attention kernel for 8 TRN2 NeuronCores.

Problem: hidden[2,2048,1024] -> QKV proj (16 heads, hd=64) -> softmax
attention -> out proj. f32 I/O, bf16 tensor-engine compute.

Sharding: sequence-parallel. Flattened rows [4096, 1024] split into 8
chunks of 512 rows; cores 0-3 own batch 0, cores 4-7 batch 1. Each core
projects K^T for its own chunk and AllGathers it within its 4-core
batch group (hidden under compute); V is projected redundantly for the
full batch on every core (cheaper than a second, serialized AllGather),
written directly into SBUF. Q^T is local. Each core then runs full
16-head attention + output projection for its 512 rows; outputs are
disjoint row blocks concatenated on the host — no reduction collective.

Layouts (all compute in [dims, seq] "transposed" form so matmul
contractions sit on the partition axis):
  hT_own [1024, 512], hT_full [1024, 2048] (host-transposed, bf16)
  kT [1024, 512] -> AllGather -> KTg [4096, 512]
  scoresT [keys, q] per head; probsT = exp(scoresT/8) (no max
    subtraction: |scores| <~ 2 by construction, softmax shift-invariant)
  ctxT accumulates via lhsT = [V_h | ones]: psum [65, 512], row 64 =
    softmax denominator; normalize = approx-reciprocal + gpsimd
    partition-broadcast + DVE multiply
  out [512, 1024] via lhsT = ctxT tiles, rhs = host-reordered Wo
"""

import numpy as np
import ml_dtypes

B, S, D, H, HD = 2, 2048, 1024, 16, 64
N_CORES = 8
ROWS = (B * S) // N_CORES          # 512 query rows per core
GROUP = 4                          # cores per batch group
P = 128
KT = D // P                        # 8 contraction tiles over hidden dim
KEYT = S // P                      # 16 key tiles per batch
HA = HD + 1                        # head slot width in v_aug

_CACHE: dict = {}

bf16 = ml_dtypes.bfloat16


def _build_graph():
    import concourse.mybir as mybir
    import concourse.tile as tile
    from concourse import bacc
    from contextlib import ExitStack

    dt = mybir.dt
    F32, BF16 = dt.float32, dt.bfloat16
    AF = mybir.ActivationFunctionType

    nc = bacc.Bacc("TRN2", target_bir_lowering=False, debug=False,
                   enable_asserts=False, num_devices=N_CORES)

    hT = nc.dram_tensor("hT", [D, ROWS], BF16, kind="ExternalInput").ap()
    hTf = nc.dram_tensor("hTf", [D, S], BF16, kind="ExternalInput").ap()
    wq = nc.dram_tensor("wq", [D, D], BF16, kind="ExternalInput").ap()
    wk = nc.dram_tensor("wk", [D, D], BF16, kind="ExternalInput").ap()
    wv = nc.dram_tensor("wv", [D, D], BF16, kind="ExternalInput").ap()
    wo = nc.dram_tensor("wo", [HD, H * D], BF16, kind="ExternalInput").ap()
    bvb = nc.dram_tensor("bvb", [P, D], BF16, kind="ExternalInput").ap()
    bob = nc.dram_tensor("bob", [P, D], BF16, kind="ExternalInput").ap()
    bqk = nc.dram_tensor("bqk", [P, 2 * KT], F32, kind="ExternalInput").ap()
    out = nc.dram_tensor("out", [ROWS, D], F32, kind="ExternalOutput").ap()

    with tile.TileContext(nc) as tc, ExitStack() as top:
        dram = top.enter_context(tc.tile_pool(name="dram", bufs=1, space="DRAM"))
        pers = top.enter_context(tc.tile_pool(name="pers", bufs=1))
        attn = top.enter_context(tc.tile_pool(name="attn", bufs=1))

        kb = dram.tile([D, ROWS], BF16)                 # local kT bounce
        KTg = dram.tile([GROUP * D, ROWS], BF16)        # gathered kT

        ones_full = pers.tile([P, P], BF16)
        nc.vector.memset(ones_full[:], 1.0)
        bqk_sb = pers.tile([P, 2 * KT], F32)
        nc.sync.dma_start(bqk_sb[:], bqk[:])
        bvb_sb = pers.tile([P, D], BF16)
        nc.sync.dma_start(bvb_sb[:], bvb[:])
        bob_sb = pers.tile([P, D], BF16)
        nc.sync.dma_start(bob_sb[:], bob[:])
        # per-head q^T slots, zero-padded on the other head's 64 partitions
        # so score matmuls can contract over the full 128 partitions (keeps
        # the PE in 128x128 mode -> no tiling-mode drains)
        qT_sb = pers.tile([P, H * ROWS], BF16)
        nc.vector.memset(qT_sb[:], 0.0)

        kt_sb = attn.tile([P, 4 * KT * ROWS], BF16)     # gathered K^T
        v_aug = attn.tile([P, KEYT * H * HA], BF16)     # [V_h | 1] slots
        nc.gpsimd.memset(v_aug[:], 1.0)
        ctx_sb = attn.tile([HD, H * ROWS], BF16)        # normalized ctx^T

        with ExitStack() as proj:
            wpool = proj.enter_context(tc.tile_pool(name="wpool", bufs=1))
            epool = proj.enter_context(tc.tile_pool(name="epool", bufs=3))
            ps_proj = proj.enter_context(
                tc.tile_pool(name="ps_proj", bufs=3, space="PSUM"))

            # input DMAs, most-urgent first
            wk_sb = wpool.tile([P, KT * D], BF16)
            nc.sync.dma_start(
                wk_sb[:].rearrange("p (k f) -> p k f", f=D),
                wk[:].rearrange("(k p) f -> p k f", p=P))
            hT_sb = wpool.tile([P, KT * ROWS], BF16)
            nc.sync.dma_start(
                hT_sb[:].rearrange("p (k f) -> p k f", f=ROWS),
                hT[:].rearrange("(k p) f -> p k f", p=P))
            wv_sb = wpool.tile([P, KT * D], BF16)
            nc.sync.dma_start(
                wv_sb[:].rearrange("p (k f) -> p k f", f=D),
                wv[:].rearrange("(k p) f -> p k f", p=P))
            hTf_sb = wpool.tile([P, KT * S], BF16)
            for kk in range(4):
                nc.sync.dma_start(
                    hTf_sb[:, kk * 2 * S:(kk + 1) * 2 * S]
                    .rearrange("p (k f) -> p k f", f=S),
                    hTf[kk * 2 * P:(kk + 1) * 2 * P, :]
                    .rearrange("(k p) f -> p k f", p=P))
            wq_sb = wpool.tile([P, KT * D], BF16)
            nc.sync.dma_start(
                wq_sb[:].rearrange("p (k f) -> p k f", f=D),
                wq[:].rearrange("(k p) f -> p k f", p=P))

            # k^T projection -> kb (bias bk folded into eviction)
            for m in range(KT):
                ps = ps_proj.tile([P, ROWS], F32, name="ps")
                for k in range(KT):
                    nc.tensor.matmul(
                        ps[:],
                        wk_sb[:, k * D + m * P: k * D + (m + 1) * P],
                        hT_sb[:, k * ROWS:(k + 1) * ROWS],
                        start=(k == 0), stop=(k == KT - 1))
                ev = epool.tile([P, ROWS], BF16, name="ev")
                nc.scalar.activation(ev[:], ps[:], AF.Identity,
                                     bias=bqk_sb[:, KT + m: KT + m + 1])
                nc.sync.dma_start(kb[m * P:(m + 1) * P, :], ev[:])

            nc.gpsimd.collective_compute(
                "AllGather", mybir.AluOpType.bypass,
                replica_groups=[[0, 1, 2, 3], [4, 5, 6, 7]],
                ins=[kb.opt()], outs=[KTg.opt()])

            # full-batch V projection, evicted straight into v_aug slots
            # (bias bv added during eviction via host-broadcast tile)
            for mk in range(KEYT):
                for n in range(2):
                    ps = ps_proj.tile([P, 512], F32, name="ps")
                    for k in range(KT):
                        nc.tensor.matmul(
                            ps[:],
                            hTf_sb[:, k * S + mk * P: k * S + (mk + 1) * P],
                            wv_sb[:, k * D + n * 512: k * D + (n + 1) * 512],
                            start=(k == 0), stop=(k == KT - 1))
                    base = (mk * H + n * 8) * HA
                    nc.vector.tensor_add(
                        v_aug[:, base: base + 8 * HA]
                        .rearrange("p (h a) -> p h a", a=HA)[:, :, 0:HD],
                        ps[:].rearrange("p (h d) -> p h d", d=HD),
                        bvb_sb[:, n * 512:(n + 1) * 512]
                        .rearrange("p (h d) -> p h d", d=HD))

            # q^T projection -> per-head zero-padded slots (bias via ACT)
            for m in range(KT):
                ps = ps_proj.tile([P, ROWS], F32, name="ps")
                for k in range(KT):
                    nc.tensor.matmul(
                        ps[:],
                        wq_sb[:, k * D + m * P: k * D + (m + 1) * P],
                        hT_sb[:, k * ROWS:(k + 1) * ROWS],
                        start=(k == 0), stop=(k == KT - 1))
                for hh in (0, 1):
                    h = 2 * m + hh
                    po = hh * HD
                    nc.scalar.activation(
                        qT_sb[po:po + HD, h * ROWS:(h + 1) * ROWS],
                        ps[po:po + HD, :], AF.Identity,
                        bias=bqk_sb[po:po + HD, m:m + 1])

        # gathered K^T into SBUF
        for r in range(GROUP):
            nc.sync.dma_start(
                kt_sb[:, r * KT * ROWS:(r + 1) * KT * ROWS]
                .rearrange("p (t f) -> p t f", f=ROWS),
                KTg[r * D:(r + 1) * D, :].rearrange("(t p) f -> p t f", p=P))

        with ExitStack() as att:
            late = att.enter_context(tc.tile_pool(name="late", bufs=1))
            wo_sb = late.tile([HD, H * D], BF16)
            nc.sync.dma_start(wo_sb[:], wo[:])

            probs = att.enter_context(tc.tile_pool(name="probs", bufs=12))
            rpool = att.enter_context(tc.tile_pool(name="rpool", bufs=2))
            bpool = att.enter_context(tc.tile_pool(name="bpool", bufs=2))

            with ExitStack() as attp:
                ps_s = attp.enter_context(
                    tc.tile_pool(name="ps_s", bufs=2, space="PSUM"))
                ps_ctx = attp.enter_context(
                    tc.tile_pool(name="ps_ctx", bufs=3, space="PSUM"))
                ps_b = attp.enter_context(
                    tc.tile_pool(name="ps_b", bufs=1, space="PSUM"))

                # software-pipelined attention: head pairs (p), u-steps of
                # two key tiles; ctx/exp consumption lags scores by LAG_U.
                U = KEYT // 2
                LAG_U = 4
                pend = {}
                psc = {}

                norm_q = []

                def emit_scores(p, u):
                    # full-128 contraction: kt tile holds both heads' dims,
                    # qT slot is zero on the other head's partitions
                    tiles = []
                    for hh in (0, 1):
                        tiles.append(ps_s.tile([P, 2 * ROWS], F32,
                                               name="ps_sc"))
                    for half in (0, 1):
                        t = 2 * u + half
                        r, m = t // (KEYT // GROUP), t % (KEYT // GROUP)
                        for hh in (0, 1):
                            h = 2 * p + hh
                            jt = h // 2
                            nc.tensor.matmul(
                                tiles[hh][:, half * ROWS:(half + 1) * ROWS],
                                kt_sb[:, (r * KT + jt) * ROWS + m * P:
                                      (r * KT + jt) * ROWS + (m + 1) * P],
                                qT_sb[:, h * ROWS:(h + 1) * ROWS],
                                start=True, stop=True)
                    for hh in (0, 1):
                        pt = probs.tile([P, 2 * ROWS], BF16, name="pt")
                        nc.scalar.activation(pt[:], tiles[hh][:], AF.Exp,
                                             scale=0.125)
                        pend[(2 * p + hh, u)] = pt

                def emit_ctx(G, p, u):
                    for hh in (0, 1):
                        h = 2 * p + hh
                        if u == 0:
                            psc[h] = ps_ctx.tile([HA, ROWS], F32, name="ps_c")
                        pt = pend.pop((h, u))
                        for half in (0, 1):
                            t = 2 * u + half
                            off = (t * H + h) * HA
                            nc.tensor.matmul(
                                psc[h][:], v_aug[:, off: off + HA],
                                pt[:, half * ROWS:(half + 1) * ROWS],
                                start=(t == 0), stop=(t == KEYT - 1))
                        if u == U - 1:
                            # reciprocal starts now (DVE); the dependent PE
                            # broadcast-matmul is deferred NORM_LAG G-steps
                            # so the PE never waits on the reciprocal
                            ps_c = psc.pop(h)
                            rt = rpool.tile([P, ROWS], BF16, name="rt")
                            nc.vector.memset(rt[:], 0.0)
                            with nc.allow_low_precision("softmax denom recip"):
                                nc.vector.reciprocal(
                                    rt[HD:HD + 1, :], ps_c[HD:HD + 1, :])
                            norm_q.append((G, h, ps_c, rt))

                def emit_norm():
                    # rt is zero except the denominator row, so an all-ones
                    # 128x128 stationary broadcasts 1/denom to all partitions
                    # without leaving 128x128 PE mode
                    _, h, ps_c, rt = norm_q.pop(0)
                    psb = ps_b.tile([P, ROWS], F32, name="psb")
                    nc.tensor.matmul(psb[:], ones_full[:], rt[:],
                                     start=True, stop=True)
                    sbb = bpool.tile([HD, ROWS], BF16, name="sbb")
                    nc.vector.tensor_copy(sbb[:], psb[0:HD, :])
                    nc.vector.tensor_mul(
                        ctx_sb[:, h * ROWS:(h + 1) * ROWS],
                        ps_c[0:HD, :], sbb[:])

                NP = H // 2
                NORM_LAG = 2
                for G in range(NP * U + LAG_U + NORM_LAG + 1):
                    # pop pending normalizes first so their ps_ctx slots free
                    # before this G-step's emit_ctx may allocate new ones
                    while norm_q and G - norm_q[0][0] >= NORM_LAG:
                        emit_norm()
                    if G < NP * U:
                        emit_scores(*divmod(G, U))
                    if LAG_U <= G < NP * U + LAG_U:
                        emit_ctx(G, *divmod(G - LAG_U, U))

            with ExitStack() as outp_s:
                ps_o = outp_s.enter_context(
                    tc.tile_pool(name="ps_o", bufs=2, space="PSUM"))
                opool = outp_s.enter_context(tc.tile_pool(name="opool", bufs=3))
                for m in range(ROWS // P):
                    for n in range(2):
                        ps = ps_o.tile([P, 512], F32, name="ps_out")
                        for h in range(H):
                            nc.tensor.matmul(
                                ps[:],
                                ctx_sb[:, h * ROWS + m * P:
                                       h * ROWS + (m + 1) * P],
                                wo_sb[:, h * D + n * 512: h * D + (n + 1) * 512],
                                start=(h == 0), stop=(h == H - 1))
                        ot = opool.tile([P, 512], F32, name="ot")
                        nc.vector.tensor_add(
                            ot[:], ps[:], bob_sb[:, n * 512:(n + 1) * 512])
                        nc.sync.dma_start(
                            out[m * P:(m + 1) * P, n * 512:(n + 1) * 512],
                            ot[:])

    nc.compile()
    return nc


def _prep_inputs(hidden_states, Wq, bq, Wk, bk, Wv, bv, Wo, bo):
    hs = np.asarray(hidden_states, np.float32).reshape(B * S, D)
    wq = np.asarray(Wq, np.float32).astype(bf16)
    wk = np.asarray(Wk, np.float32).astype(bf16)
    wv = np.asarray(Wv, np.float32).astype(bf16)
    wo = (np.asarray(Wo, np.float32).reshape(H, HD, D)
          .transpose(1, 0, 2).reshape(HD, H * D).astype(bf16))
    bvb = np.ascontiguousarray(np.broadcast_to(
        np.asarray(bv, np.float32)[None], (P, D))).astype(bf16)
    bob = np.ascontiguousarray(np.broadcast_to(
        np.asarray(bo, np.float32)[None], (P, D))).astype(bf16)
    bqk = np.ascontiguousarray(np.concatenate(
        [np.asarray(bq, np.float32).reshape(KT, P).T,
         np.asarray(bk, np.float32).reshape(KT, P).T], 1).astype(np.float32))
    hTf = [np.ascontiguousarray(hs[b * S:(b + 1) * S].T).astype(bf16)
           for b in range(B)]
    in_maps = []
    for c in range(N_CORES):
        hT = np.ascontiguousarray(
            hs[c * ROWS:(c + 1) * ROWS].T).astype(bf16)
        in_maps.append({"hT": hT, "hTf": hTf[c // GROUP], "wq": wq, "wk": wk,
                        "wv": wv, "wo": wo, "bvb": bvb, "bob": bob,
                        "bqk": bqk})
    return in_maps


def _run(inputs, trace=False):
    from concourse import bass_utils
    if "nc" not in _CACHE:
        _CACHE["nc"] = _build_graph()
    nc = _CACHE["nc"]
    in_maps = _prep_inputs(**inputs)
    res = bass_utils.run_bass_kernel_spmd(
        nc, in_maps, core_ids=list(range(N_CORES)), trace=trace)
    full = np.concatenate([res.results[c]["out"] for c in range(N_CORES)],
                          axis=0).reshape(B, S, D).astype(np.float32)
    return full, res


def kernel(**inputs) -> np.ndarray:
    full, _ = _run(inputs, trace=False)
    return full


# revision 31
# speedup vs baseline: 1.6618x; 1.1065x over previous
"""Distributed multi-head attention kernel for 8 TRN2 NeuronCores.

Problem: hidden[2,2048,1024] -> QKV proj (16 heads, hd=64) -> softmax
attention -> out proj. f32 I/O, bf16 tensor-engine compute.

Sharding: sequence-parallel. Flattened rows [4096, 1024] split into 8
chunks of 512 rows; cores 0-3 own batch 0, cores 4-7 batch 1. Each core
projects K^T for its own chunk and AllGathers it within its 4-core
batch group (hidden under compute); V is projected redundantly for the
full batch on every core (cheaper than a second, serialized AllGather),
written directly into SBUF. Q^T is local. Each core then runs full
16-head attention + output projection for its 512 rows; outputs are
disjoint row blocks concatenated on the host — no reduction collective.

Layouts (all compute in [dims, seq] "transposed" form so matmul
contractions sit on the partition axis):
  hT_own [1024, 512], hT_full [1024, 2048] (host-transposed, bf16)
  kT [1024, 512] -> AllGather -> KTg [4096, 512]
  scoresT [keys, q] per head; probsT = exp(scoresT/8) (no max
    subtraction: |scores| <~ 2 by construction, softmax shift-invariant)
  ctxT accumulates via lhsT = [V_h | ones]: psum [65, 512], row 64 =
    softmax denominator; normalize = approx-reciprocal + gpsimd
    partition-broadcast + DVE multiply
  out [512, 1024] via lhsT = ctxT tiles, rhs = host-reordered Wo
"""

import numpy as np
import ml_dtypes

B, S, D, H, HD = 2, 2048, 1024, 16, 64
N_CORES = 8
ROWS = (B * S) // N_CORES          # 512 query rows per core
GROUP = 4                          # cores per batch group
P = 128
KT = D // P                        # 8 contraction tiles over hidden dim
KEYT = S // P                      # 16 key tiles per batch
HA = HD + 1                        # head slot width in v_aug

_CACHE: dict = {}

bf16 = ml_dtypes.bfloat16


def _build_graph():
    import concourse.mybir as mybir
    import concourse.tile as tile
    from concourse import bacc
    from contextlib import ExitStack

    dt = mybir.dt
    F32, BF16 = dt.float32, dt.bfloat16
    AF = mybir.ActivationFunctionType

    nc = bacc.Bacc("TRN2", target_bir_lowering=False, debug=False,
                   enable_asserts=False, num_devices=N_CORES)

    hT = nc.dram_tensor("hT", [D, ROWS], BF16, kind="ExternalInput").ap()
    hTf = nc.dram_tensor("hTf", [D, S], BF16, kind="ExternalInput").ap()
    wq = nc.dram_tensor("wq", [D, D], BF16, kind="ExternalInput").ap()
    wk = nc.dram_tensor("wk", [D, D], BF16, kind="ExternalInput").ap()
    wv = nc.dram_tensor("wv", [D, D], BF16, kind="ExternalInput").ap()
    wo = nc.dram_tensor("wo", [D, D], BF16, kind="ExternalInput").ap()
    bvb = nc.dram_tensor("bvb", [P, D], BF16, kind="ExternalInput").ap()
    bob = nc.dram_tensor("bob", [P, D], BF16, kind="ExternalInput").ap()
    bqk = nc.dram_tensor("bqk", [P, 2 * KT], F32, kind="ExternalInput").ap()
    out = nc.dram_tensor("out", [ROWS, D], F32, kind="ExternalOutput").ap()

    with tile.TileContext(nc) as tc, ExitStack() as top:
        dram = top.enter_context(tc.tile_pool(name="dram", bufs=1, space="DRAM"))
        pers = top.enter_context(tc.tile_pool(name="pers", bufs=1))
        attn = top.enter_context(tc.tile_pool(name="attn", bufs=1))

        kb = dram.tile([D, ROWS], BF16)                 # local kT bounce
        KTg = dram.tile([GROUP * D, ROWS], BF16)        # gathered kT

        ones_full = pers.tile([P, P], BF16)
        nc.vector.memset(ones_full[:], 1.0)
        bqk_sb = pers.tile([P, 2 * KT], F32)
        nc.sync.dma_start(bqk_sb[:], bqk[:])
        bvb_sb = pers.tile([P, D], BF16)
        nc.sync.dma_start(bvb_sb[:], bvb[:])
        bob_sb = pers.tile([P, D], BF16)
        nc.sync.dma_start(bob_sb[:], bob[:])
        # per-head q^T slots, zero-padded on the other head's 64 partitions
        # so score matmuls can contract over the full 128 partitions (keeps
        # the PE in 128x128 mode -> no tiling-mode drains)
        qT_sb = pers.tile([P, H * ROWS], BF16)
        nc.vector.memset(qT_sb[:], 0.0)

        kt_sb = attn.tile([P, 4 * KT * ROWS], BF16)     # gathered K^T
        v_aug = attn.tile([P, KEYT * H * HA], BF16)     # [V_h | 1] slots
        nc.gpsimd.memset(v_aug[:], 1.0)
        # pair-packed normalized ctx^T: head 2j on partitions 0-63 of pair
        # slot j, head 2j+1 on partitions 64-127 (odd heads arrive via a
        # cross-partition SBUF DMA from ctx_odd)
        ctx_pair = attn.tile([P, (H // 2) * ROWS], BF16)
        ctx_odd = attn.tile([HD, (H // 2) * ROWS], BF16)
        # persistent broadcast staging: rt row 64 = f32 denom reciprocal,
        # rtb = bf16 copy, all other rows stay zero forever
        rt2 = [attn.tile([P, ROWS], F32, name=f"rt2_{i}") for i in range(2)]
        rtb2 = [attn.tile([P, ROWS], BF16, name=f"rtb2_{i}") for i in range(2)]
        for i in range(2):
            nc.vector.memset(rtb2[i][:], 0.0)

        with ExitStack() as proj:
            wpool = proj.enter_context(tc.tile_pool(name="wpool", bufs=1))
            epool = proj.enter_context(tc.tile_pool(name="epool", bufs=3))
            ps_proj = proj.enter_context(
                tc.tile_pool(name="ps_proj", bufs=3, space="PSUM"))

            # input DMAs, most-urgent first
            wk_sb = wpool.tile([P, KT * D], BF16)
            nc.sync.dma_start(
                wk_sb[:].rearrange("p (k f) -> p k f", f=D),
                wk[:].rearrange("(k p) f -> p k f", p=P))
            hT_sb = wpool.tile([P, KT * ROWS], BF16)
            nc.sync.dma_start(
                hT_sb[:].rearrange("p (k f) -> p k f", f=ROWS),
                hT[:].rearrange("(k p) f -> p k f", p=P))
            wv_sb = wpool.tile([P, KT * D], BF16)
            nc.sync.dma_start(
                wv_sb[:].rearrange("p (k f) -> p k f", f=D),
                wv[:].rearrange("(k p) f -> p k f", p=P))
            hTf_sb = wpool.tile([P, KT * S], BF16)
            for kk in range(4):
                nc.sync.dma_start(
                    hTf_sb[:, kk * 2 * S:(kk + 1) * 2 * S]
                    .rearrange("p (k f) -> p k f", f=S),
                    hTf[kk * 2 * P:(kk + 1) * 2 * P, :]
                    .rearrange("(k p) f -> p k f", p=P))
            wq_sb = wpool.tile([P, KT * D], BF16)
            nc.sync.dma_start(
                wq_sb[:].rearrange("p (k f) -> p k f", f=D),
                wq[:].rearrange("(k p) f -> p k f", p=P))

            # k^T projection -> kb (bias bk folded into eviction)
            for m in range(KT):
                ps = ps_proj.tile([P, ROWS], F32, name="ps")
                for k in range(KT):
                    nc.tensor.matmul(
                        ps[:],
                        wk_sb[:, k * D + m * P: k * D + (m + 1) * P],
                        hT_sb[:, k * ROWS:(k + 1) * ROWS],
                        start=(k == 0), stop=(k == KT - 1))
                ev = epool.tile([P, ROWS], BF16, name="ev")
                nc.scalar.activation(ev[:], ps[:], AF.Identity,
                                     bias=bqk_sb[:, KT + m: KT + m + 1])
                nc.sync.dma_start(kb[m * P:(m + 1) * P, :], ev[:])

            nc.gpsimd.collective_compute(
                "AllGather", mybir.AluOpType.bypass,
                replica_groups=[[0, 1, 2, 3], [4, 5, 6, 7]],
                ins=[kb.opt()], outs=[KTg.opt()])

            # full-batch V projection, evicted straight into v_aug slots
            # (bias bv added during eviction via host-broadcast tile)
            for mk in range(KEYT):
                for n in range(2):
                    ps = ps_proj.tile([P, 512], F32, name="ps")
                    for k in range(KT):
                        nc.tensor.matmul(
                            ps[:],
                            hTf_sb[:, k * S + mk * P: k * S + (mk + 1) * P],
                            wv_sb[:, k * D + n * 512: k * D + (n + 1) * 512],
                            start=(k == 0), stop=(k == KT - 1))
                    base = (mk * H + n * 8) * HA
                    nc.vector.tensor_add(
                        v_aug[:, base: base + 8 * HA]
                        .rearrange("p (h a) -> p h a", a=HA)[:, :, 0:HD],
                        ps[:].rearrange("p (h d) -> p h d", d=HD),
                        bvb_sb[:, n * 512:(n + 1) * 512]
                        .rearrange("p (h d) -> p h d", d=HD))

            # q^T projection -> per-head zero-padded slots (bias via ACT)
            for m in range(KT):
                ps = ps_proj.tile([P, ROWS], F32, name="ps")
                for k in range(KT):
                    nc.tensor.matmul(
                        ps[:],
                        wq_sb[:, k * D + m * P: k * D + (m + 1) * P],
                        hT_sb[:, k * ROWS:(k + 1) * ROWS],
                        start=(k == 0), stop=(k == KT - 1))
                for hh in (0, 1):
                    h = 2 * m + hh
                    po = hh * HD
                    nc.scalar.activation(
                        qT_sb[po:po + HD, h * ROWS:(h + 1) * ROWS],
                        ps[po:po + HD, :], AF.Identity,
                        bias=bqk_sb[po:po + HD, m:m + 1])

        # gathered K^T into SBUF
        for r in range(GROUP):
            nc.sync.dma_start(
                kt_sb[:, r * KT * ROWS:(r + 1) * KT * ROWS]
                .rearrange("p (t f) -> p t f", f=ROWS),
                KTg[r * D:(r + 1) * D, :].rearrange("(t p) f -> p t f", p=P))

        with ExitStack() as att:
            late = att.enter_context(tc.tile_pool(name="late", bufs=1))
            wo_sb = late.tile([P, KT * D], BF16)
            nc.sync.dma_start(
                wo_sb[:].rearrange("p (k f) -> p k f", f=D),
                wo[:].rearrange("(k p) f -> p k f", p=P))

            probs = att.enter_context(tc.tile_pool(name="probs", bufs=12))
            bpool = att.enter_context(tc.tile_pool(name="bpool", bufs=2))

            with ExitStack() as attp:
                ps_s = attp.enter_context(
                    tc.tile_pool(name="ps_s", bufs=2, space="PSUM"))
                ps_ctx = attp.enter_context(
                    tc.tile_pool(name="ps_ctx", bufs=3, space="PSUM"))
                ps_b = attp.enter_context(
                    tc.tile_pool(name="ps_b", bufs=1, space="PSUM"))

                # software-pipelined attention: head pairs (p), u-steps of
                # two key tiles; ctx/exp consumption lags scores by LAG_U.
                U = KEYT // 2
                LAG_U = 4
                pend = {}
                psc = {}

                norm_q = []

                def emit_scores(p, u):
                    # full-128 contraction: kt tile holds both heads' dims,
                    # qT slot is zero on the other head's partitions
                    tiles = []
                    for hh in (0, 1):
                        tiles.append(ps_s.tile([P, 2 * ROWS], F32,
                                               name="ps_sc"))
                    for half in (0, 1):
                        t = 2 * u + half
                        r, m = t // (KEYT // GROUP), t % (KEYT // GROUP)
                        for hh in (0, 1):
                            h = 2 * p + hh
                            jt = h // 2
                            nc.tensor.matmul(
                                tiles[hh][:, half * ROWS:(half + 1) * ROWS],
                                kt_sb[:, (r * KT + jt) * ROWS + m * P:
                                      (r * KT + jt) * ROWS + (m + 1) * P],
                                qT_sb[:, h * ROWS:(h + 1) * ROWS],
                                start=True, stop=True)
                    for hh in (0, 1):
                        pt = probs.tile([P, 2 * ROWS], BF16, name="pt")
                        nc.scalar.activation(pt[:], tiles[hh][:], AF.Exp,
                                             scale=0.125)
                        pend[(2 * p + hh, u)] = pt

                def emit_ctx(G, p, u):
                    for hh in (0, 1):
                        h = 2 * p + hh
                        if u == 0:
                            psc[h] = ps_ctx.tile([HA, ROWS], F32, name="ps_c")
                        pt = pend.pop((h, u))
                        for half in (0, 1):
                            t = 2 * u + half
                            off = (t * H + h) * HA
                            nc.tensor.matmul(
                                psc[h][:], v_aug[:, off: off + HA],
                                pt[:, half * ROWS:(half + 1) * ROWS],
                                start=(t == 0), stop=(t == KEYT - 1))
                        if u == U - 1:
                            # approx-reciprocal starts now (DVE); the
                            # dependent PE broadcast-matmul is deferred
                            # NORM_LAG G-steps so the PE never waits on it
                            ps_c = psc.pop(h)
                            rtb = rtb2[h % 2]
                            with nc.allow_low_precision("softmax denom recip"):
                                nc.vector.reciprocal(
                                    rtb[HD:HD + 1, :], ps_c[HD:HD + 1, :])
                            norm_q.append((G, h, ps_c, rtb))

                def emit_norm():
                    # rtb is zero except the denominator row, so an all-ones
                    # 128x128 stationary broadcasts 1/denom to all partitions
                    # without leaving 128x128 PE mode
                    _, h, ps_c, rtb = norm_q.pop(0)
                    psb = ps_b.tile([P, ROWS], F32, name="psb")
                    nc.tensor.matmul(psb[:], ones_full[:], rtb[:],
                                     start=True, stop=True)
                    sbb = bpool.tile([HD, ROWS], BF16, name="sbb")
                    nc.vector.tensor_copy(sbb[:], psb[0:HD, :])
                    if h % 2 == 0:
                        dst = ctx_pair[0:HD, (h // 2) * ROWS:
                                       (h // 2 + 1) * ROWS]
                    else:
                        dst = ctx_odd[:, (h // 2) * ROWS:(h // 2 + 1) * ROWS]
                    nc.vector.tensor_mul(dst, ps_c[0:HD, :], sbb[:])
                    if h % 2 == 1:
                        # cross-partition shuffle: odd head -> partitions
                        # 64-127 of the pair slot (DMA is not lane-locked)
                        nc.sync.dma_start(
                            ctx_pair[HD:P, (h // 2) * ROWS:
                                     (h // 2 + 1) * ROWS],
                            ctx_odd[:, (h // 2) * ROWS:(h // 2 + 1) * ROWS])

                NP = H // 2
                NORM_LAG = 1
                for G in range(NP * U + LAG_U + NORM_LAG + 1):
                    # pop pending normalizes first so their ps_ctx slots free
                    # before this G-step's emit_ctx may allocate new ones
                    while norm_q and G - norm_q[0][0] >= NORM_LAG:
                        emit_norm()
                    if G < NP * U:
                        emit_scores(*divmod(G, U))
                    if LAG_U <= G < NP * U + LAG_U:
                        emit_ctx(G, *divmod(G - LAG_U, U))

            with ExitStack() as outp_s:
                ps_o = outp_s.enter_context(
                    tc.tile_pool(name="ps_o", bufs=2, space="PSUM"))
                opool = outp_s.enter_context(tc.tile_pool(name="opool", bufs=3))
                for m in range(ROWS // P):
                    for n in range(2):
                        ps = ps_o.tile([P, 512], F32, name="ps_out")
                        for j in range(H // 2):
                            nc.tensor.matmul(
                                ps[:],
                                ctx_pair[:, j * ROWS + m * P:
                                         j * ROWS + (m + 1) * P],
                                wo_sb[:, j * D + n * 512: j * D + (n + 1) * 512],
                                start=(j == 0), stop=(j == H // 2 - 1))
                        ot = opool.tile([P, 512], F32, name="ot")
                        nc.vector.tensor_add(
                            ot[:], ps[:], bob_sb[:, n * 512:(n + 1) * 512])
                        nc.sync.dma_start(
                            out[m * P:(m + 1) * P, n * 512:(n + 1) * 512],
                            ot[:])

    nc.compile()
    return nc


def _prep_inputs(hidden_states, Wq, bq, Wk, bk, Wv, bv, Wo, bo):
    hs = np.asarray(hidden_states, np.float32).reshape(B * S, D)
    wq = np.asarray(Wq, np.float32).astype(bf16)
    wk = np.asarray(Wk, np.float32).astype(bf16)
    wv = np.asarray(Wv, np.float32).astype(bf16)
    wo = np.asarray(Wo, np.float32).astype(bf16)
    bvb = np.ascontiguousarray(np.broadcast_to(
        np.asarray(bv, np.float32)[None], (P, D))).astype(bf16)
    bob = np.ascontiguousarray(np.broadcast_to(
        np.asarray(bo, np.float32)[None], (P, D))).astype(bf16)
    bqk = np.ascontiguousarray(np.concatenate(
        [np.asarray(bq, np.float32).reshape(KT, P).T,
         np.asarray(bk, np.float32).reshape(KT, P).T], 1).astype(np.float32))
    hTf = [np.ascontiguousarray(hs[b * S:(b + 1) * S].T).astype(bf16)
           for b in range(B)]
    in_maps = []
    for c in range(N_CORES):
        hT = np.ascontiguousarray(
            hs[c * ROWS:(c + 1) * ROWS].T).astype(bf16)
        in_maps.append({"hT": hT, "hTf": hTf[c // GROUP], "wq": wq, "wk": wk,
                        "wv": wv, "wo": wo, "bvb": bvb, "bob": bob,
                        "bqk": bqk})
    return in_maps


def _run(inputs, trace=False):
    from concourse import bass_utils
    if "nc" not in _CACHE:
        _CACHE["nc"] = _build_graph()
    nc = _CACHE["nc"]
    in_maps = _prep_inputs(**inputs)
    res = bass_utils.run_bass_kernel_spmd(
        nc, in_maps, core_ids=list(range(N_CORES)), trace=trace)
    full = np.concatenate([res.results[c]["out"] for c in range(N_CORES)],
                          axis=0).reshape(B, S, D).astype(np.float32)
    return full, res


def kernel(**inputs) -> np.ndarray:
    full, _ = _run(inputs, trace=False)
    return full
